# revision 17
# baseline (speedup 1.0000x reference)
"""Trainium2 8-core kernel for nn_AnalyticFlow (retrieval_knn) — fp8 DoubleRow.

Math (reference):
    z[b,p] = alpha_b * (x_b . g_p)      (softmax logits; the per-row quadratic
             term is dropped exactly, and the tn^2*||g||^2 bias term is dropped
             because its across-p spread is <= inv_var*tn^2*std(||g||^2) ~ 4e-5
             in logit units -- invisible at the 2e-2 gate)
    w      = softmax_p(z)
    out    = (1/(1-tn_b)) * (sum_p w[b,p] g_p - x_b)

Device strategy (SPMD over 8 NeuronCores, P sharded 6250/core):
    - All matmuls are fp8e4m3 DoubleRow (157 TF/s): 3D APs [128, 2, N] whose
      middle dim holds two adjacent 128-row K-subtiles.
    - Phase 1 (over p-windows of 512): z[b,p] = x''^T.T @ G'^T with
      x'' = (alpha*XSCALE)*x as the stationary operand (2^9 prescale keeps fp8
      out of the subnormal range); ScalarE exp(scale=1/XSCALE) emits E fp8 and
      accum_out accumulates the softmax denominator s per batch row (the final
      partial window is split so pad columns never pollute s); PE transposes
      E 128x128 tiles into the resident E^T [p, b] buffer.
    - Phase 2 (mm2) runs in two d-column halves x two p-pair sets:
      W[b,d] += E^T-pair.T @ G_nat-pair accumulates a full pair-set per PSUM
      bank; the first d-half's ReduceScatter is issued while the second half
      computes, so only the second RS is exposed as tail.
    - Each core epilogues its own 32 batch rows in f32 (out ~= -x/(1-tn), so
      f32 epilogue keeps overall rel-err ~2e-4 despite fp8 matmuls).
"""

import math

import numpy as np
import ml_dtypes

import concourse.bass as bass
import concourse.bacc as bacc
import concourse.tile as tile
import concourse.mybir as mybir
from concourse import bass_utils

FP8 = mybir.dt.float8e4
F32 = mybir.dt.float32
NP_FP8 = mybir.dt.np(FP8)

T_SCHEDULE = 999.0
N_CORES = 8
XSCALE = 512.0  # power-of-2 prescale on x'' so fp8 values stay normal-range
PW = 512        # p-window width for mm1


class Cfg:
    def __init__(self, B=256, D=3072, P=50000):
        assert B % 128 == 0 and B % N_CORES == 0
        assert D % 512 == 0 and D % 256 == 0
        assert P % N_CORES == 0
        self.B = B
        self.D = D
        self.P = P
        self.PSH = P // N_CORES                       # db rows per core
        self.NW = math.ceil(self.PSH / PW)            # mm1 p-windows
        self.PCH = math.ceil(self.PSH / 256)          # p-chunk PAIRS for mm2
        self.KCH = D // 256                           # K-pairs (d-chunks / 2)
        self.NDG = D // 512                           # mm2 512-wide d-groups
        self.BC = B // 128                            # b-chunks
        self.BR = B // N_CORES                        # output rows per core
        self.NH = 2 if self.NDG % 2 == 0 else 1       # d-halves (split RS)


def build_nc(cfg: Cfg):
    nc = bacc.Bacc(
        "TRN2", target_bir_lowering=False, debug=False, num_devices=N_CORES
    )
    gT = nc.declare_dram_parameter(
        "gT", [cfg.NW, 128, cfg.KCH, 2, PW], FP8, isOutput=False
    )
    gN = nc.declare_dram_parameter(
        "gN", [cfg.PCH, 128, 2, cfg.D], FP8, isOutput=False
    )
    # mm1 stationary operand, SW-interleaved for DoubleRowSwInterleave:
    # [i, k, m, j] = x''T[(2k+j)*128+i, bchunk*128 + (127-m)]
    xT = nc.declare_dram_parameter(
        "xT", [128, cfg.KCH, cfg.B, 2], FP8, isOutput=False
    )
    xi = nc.declare_dram_parameter("xi", [cfg.BR, cfg.D], F32, isOutput=False)
    sc = nc.declare_dram_parameter("sc", [cfg.BR, 1], F32, isOutput=False)
    out = nc.declare_dram_parameter("out", [cfg.BR, cfg.D], F32, isOutput=True)

    DR = mybir.MatmulPerfMode.DoubleRow
    DRSW = mybir.MatmulPerfMode.DoubleRowSwInterleave
    EXP = mybir.ActivationFunctionType.Exp

    HC = cfg.D // cfg.NH          # columns per d-half
    HG = cfg.NDG // cfg.NH        # 512-groups per d-half
    # p-pair sets (second-level K split of mm2 so PSUM holds a full set)
    half_pairs = math.ceil(cfg.PCH / 2)
    psets = [list(range(half_pairs)), list(range(half_pairs, cfg.PCH))]
    psets = [ps for ps in psets if ps]

    ident_np = np.eye(128, dtype=NP_FP8)
    ident_dram = nc.inline_tensor(ident_np, name="ident")

    with tile.TileContext(nc) as tc:
        with (
            tc.tile_pool(name="persist", bufs=1) as pp,
            tc.tile_pool(name="gtp", bufs=3) as gtp,
            tc.tile_pool(name="gnp", bufs=2 * half_pairs + 2) as gnp,
            tc.tile_pool(name="ep", bufs=4) as epool,
            tc.tile_pool(name="stg", bufs=1) as stg,
            tc.tile_pool(name="dram", bufs=1, space="DRAM") as dram,
            tc.tile_pool(name="zps", bufs=3, space="PSUM") as zpool,
            tc.tile_pool(name="wps", bufs=2, space="PSUM") as wpool,
            tc.tile_pool(name="tps", bufs=3, space="PSUM") as tpool,
        ):
            xT_sb = pp.tile([128, cfg.KCH, cfg.B, 2], FP8)
            nc.sync.dma_start(xT_sb[:], xT[:])
            ident_sb = pp.tile([128, 128], FP8)
            nc.sync.dma_start(ident_sb[:], ident_dram[:])
            xi_sb = pp.tile([cfg.BR, cfg.D], F32)
            nc.sync.dma_start(xi_sb[:], xi[:])
            sc_sb = pp.tile([cfg.BR, 1], F32)
            nc.sync.dma_start(sc_sb[:], sc[:])

            ET = pp.tile([128, cfg.PCH, 2, cfg.B], FP8)
            s_acc = [
                pp.tile([128, 1], F32, name=f"sacc{b}", tag=f"sacc{b}")
                for b in range(cfg.BC)
            ]
            Wah = [
                pp.tile([128, HC], F32, name=f"wah{b}", tag=f"wah{b}")
                for b in range(cfg.BC)
            ]
            for b in range(cfg.BC):
                nc.vector.memset(s_acc[b][:], 0.0)

            rs_in = [
                dram.tile(
                    [cfg.B, HC + (1 if h == cfg.NH - 1 else 0)], F32,
                    name=f"rsin{h}",
                )
                for h in range(cfg.NH)
            ]
            rs_out = [
                dram.tile(
                    [cfg.BR, HC + (1 if h == cfg.NH - 1 else 0)], F32,
                    name=f"rsout{h}",
                )
                for h in range(cfg.NH)
            ]

            # ---------------- Phase 1: mm1 + exp + transposes ----------------
            for w in range(cfg.NW):
                gt_t = gtp.tile([128, cfg.KCH, 2, PW], FP8, tag="gt")
                nc.sync.dma_start(gt_t[:], gT[w])
                vc = max(0, min(cfg.PSH - w * PW, PW))
                for b in range(cfg.BC):
                    z = zpool.tile([128, PW], F32, tag="z")
                    for k in range(cfg.KCH):
                        nc.tensor.matmul(
                            z[:],
                            xT_sb[:, k, b * 128 : (b + 1) * 128, :],
                            gt_t[:, k, :, :],
                            start=(k == 0),
                            stop=(k == cfg.KCH - 1),
                            perf_mode=DRSW,
                        )
                    e_t = epool.tile([128, PW], FP8, tag="e")
                    s_part = pp.tile(
                        [128, 1], F32, name=f"sp{w}_{b}", tag="spart", bufs=4
                    )
                    if vc == PW:
                        nc.scalar.activation(
                            e_t[:], z[:], EXP, scale=1.0 / XSCALE,
                            accum_out=s_part[:],
                        )
                    else:
                        nc.scalar.activation(
                            e_t[:, :vc], z[:, :vc], EXP, scale=1.0 / XSCALE,
                            accum_out=s_part[:],
                        )
                        nc.scalar.activation(
                            e_t[:, vc:], z[:, vc:], EXP, scale=1.0 / XSCALE,
                        )
                    nc.vector.tensor_add(s_acc[b][:], s_acc[b][:], s_part[:])
                    # transpose the window's 128x128 E blocks into one PSUM
                    # tile (fp8 transpose writes with element step 2), then
                    # one batched ScalarE copy into ET
                    nv = min(PW // 128, 2 * cfg.PCH - w * (PW // 128))
                    if nv <= 0:
                        continue
                    t_ps = tpool.tile([128, PW, 2], FP8, tag="t")
                    for c in range(nv):
                        nc.tensor.transpose(
                            t_ps[:, c * 128 : (c + 1) * 128, 0],
                            e_t[:, c * 128 : (c + 1) * 128],
                            ident_sb[:],
                        )
                    src = t_ps[:, : nv * 128, 0].rearrange(
                        "p (a c) -> p a c", c=128
                    )
                    npair = (nv + 1) // 2
                    dst = ET[
                        :, 2 * w : 2 * w + npair, :, b * 128 : (b + 1) * 128
                    ].rearrange("p a b c -> p (a b) c")[:, :nv, :]
                    nc.scalar.copy(dst, src)

            # ------------- Phase 2: mm2 per d-half, split RS -------------
            for h in range(cfg.NH):
                c0 = h * HC
                for si, pset in enumerate(psets):
                    gn_tiles = {}
                    for m in pset:
                        gn_t = gnp.tile([128, 2, HC], FP8, tag="gn")
                        nc.sync.dma_start(gn_t[:], gN[m, :, :, c0 : c0 + HC])
                        gn_tiles[m] = gn_t
                    for b in range(cfg.BC):
                        for dg in range(HG):
                            wp = wpool.tile([128, 512], F32, tag="w")
                            for j, m in enumerate(pset):
                                nc.tensor.matmul(
                                    wp[:],
                                    ET[:, m, :, b * 128 : (b + 1) * 128],
                                    gn_tiles[m][
                                        :, :, dg * 512 : (dg + 1) * 512
                                    ],
                                    start=(j == 0),
                                    stop=(j == len(pset) - 1),
                                    perf_mode=DR,
                                )
                            dst = Wah[b][:, dg * 512 : (dg + 1) * 512]
                            if si == 0:
                                nc.scalar.copy(dst, wp[:])
                            else:
                                nc.vector.tensor_add(dst, dst, wp[:])
                            if si == len(psets) - 1:
                                nc.sync.dma_start(
                                    rs_in[h][
                                        b * 128 : (b + 1) * 128,
                                        dg * 512 : (dg + 1) * 512,
                                    ],
                                    dst,
                                )
                if h == cfg.NH - 1:
                    for b in range(cfg.BC):
                        nc.sync.dma_start(
                            rs_in[h][b * 128 : (b + 1) * 128, HC : HC + 1],
                            s_acc[b][:],
                        )
                nc.gpsimd.collective_compute(
                    "ReduceScatter",
                    mybir.AluOpType.add,
                    replica_groups=[list(range(N_CORES))],
                    ins=[rs_in[h].opt()],
                    outs=[rs_out[h].opt()],
                )

            # ---------------------- epilogue ----------------------
            eps = []
            for h in range(cfg.NH):
                w_ = HC + (1 if h == cfg.NH - 1 else 0)
                e = stg.tile([cfg.BR, w_], F32, name=f"eph{h}", tag=f"eph{h}")
                nc.sync.dma_start(e[:], rs_out[h][:])
                eps.append(e)
            rec = pp.tile([cfg.BR, 1], F32)
            nc.vector.reciprocal(rec[:], eps[-1][:, HC : HC + 1])
            nc.vector.tensor_mul(rec[:], rec[:], sc_sb[:])
            out_sb = pp.tile([cfg.BR, cfg.D], F32)
            for h in range(cfg.NH):
                nc.vector.scalar_tensor_tensor(
                    out_sb[:, h * HC : h * HC + HC],
                    eps[h][:, :HC],
                    rec[:],
                    xi_sb[:, h * HC : h * HC + HC],
                    op0=mybir.AluOpType.mult,
                    op1=mybir.AluOpType.subtract,
                )
            nc.sync.dma_start(out[:], out_sb[:])

    nc.compile()
    return nc


def prep_in_maps(cfg: Cfg, xt, t, gt_images):
    B, D, P = cfg.B, cfg.D, cfg.P
    x = np.asarray(xt, dtype=np.float32).reshape(B, -1)
    g = np.asarray(gt_images, dtype=np.float32).reshape(P, -1)
    t = np.asarray(t, dtype=np.float32).reshape(B)
    assert x.shape[1] == D

    tn = t / T_SCHEDULE
    inv_var = 1.0 / (2.0 * (1.0 - tn) ** 2)
    alpha = 2.0 * inv_var * tn
    inv1mtn = 1.0 / (1.0 - tn)

    # x''^T pretiled + SW-interleaved for DoubleRowSwInterleave:
    # [128, KCH, B, 2]: [i, k, bc*128+m, j] = x''T[(2k+j)*128+i, bc*128+127-m]
    xp = (x * (alpha * XSCALE)[:, None]).T  # [D, B]
    x4 = xp.reshape(cfg.KCH, 2, 128, B // 128, 128)   # [k, j, i, bc, n]
    x4 = x4[:, :, :, :, ::-1]                          # reverse cols in chunk
    xT_tiled = np.ascontiguousarray(
        x4.transpose(2, 0, 3, 4, 1).reshape(128, cfg.KCH, B, 2)
    ).astype(NP_FP8)

    PWTOT = cfg.NW * PW
    in_maps = []
    for c in range(N_CORES):
        gs = g[c * cfg.PSH : (c + 1) * cfg.PSH]
        # G'^T padded [D, PWTOT] -> [NW, 128, KCH, 2, PW]:
        # [w, i, k, j, p'] = G'T[(2k+j)*128+i, w*PW+p']
        gtp_ = np.zeros((D, PWTOT), np.float32)
        gtp_[:, : cfg.PSH] = gs.T
        gTb = np.ascontiguousarray(
            gtp_.reshape(cfg.KCH, 2, 128, cfg.NW, PW).transpose(3, 2, 0, 1, 4)
        ).astype(NP_FP8)
        # G_nat pair tiles [PCH, 128, 2, D]: [m, i, j, d] = G[(2m+j)*128+i, d]
        gn_ = np.zeros((cfg.PCH * 256, cfg.D), np.float32)
        gn_[: cfg.PSH] = gs
        gNb = np.ascontiguousarray(
            gn_.reshape(cfg.PCH, 2, 128, cfg.D).transpose(0, 2, 1, 3)
        ).astype(NP_FP8)
        rows = slice(c * cfg.BR, (c + 1) * cfg.BR)
        xi = np.ascontiguousarray(x[rows] * inv1mtn[rows, None]).astype(np.float32)
        sc = np.ascontiguousarray(inv1mtn[rows, None]).astype(np.float32)
        in_maps.append({"gT": gTb, "gN": gNb, "xT": xT_tiled, "xi": xi, "sc": sc})
    return in_maps


_NC_CACHE = {}


def _get_nc(cfg: Cfg):
    key = (cfg.B, cfg.D, cfg.P)
    if key not in _NC_CACHE:
        _NC_CACHE[key] = build_nc(cfg)
    return _NC_CACHE[key]


def kernel(xt, t, gt_images, _trace=False):
    xt = np.asarray(xt)
    cfg = Cfg(B=xt.shape[0], D=int(np.prod(xt.shape[1:])),
              P=np.asarray(gt_images).shape[0])
    nc = _get_nc(cfg)
    in_maps = prep_in_maps(cfg, xt, t, gt_images)
    res = bass_utils.run_bass_kernel_spmd(
        nc, in_maps, core_ids=list(range(N_CORES)), trace=_trace
    )
    out = np.concatenate(
        [res.results[c]["out"] for c in range(N_CORES)], axis=0
    ).astype(np.float32)
    if _trace:
        kernel.last_exec_time_ns = res.exec_time_ns
    return out.reshape(xt.shape)


# revision 19
# speedup vs baseline: 1.0066x; 1.0066x over previous
"""Trainium2 8-core kernel for nn_AnalyticFlow (retrieval_knn) — fp8 DoubleRow.

Math (reference):
    z[b,p] = alpha_b * (x_b . g_p)      (softmax logits; the per-row quadratic
             term is dropped exactly, and the tn^2*||g||^2 bias term is dropped
             because its across-p spread is <= inv_var*tn^2*std(||g||^2) ~ 4e-5
             in logit units -- invisible at the 2e-2 gate)
    w      = softmax_p(z)
    out    = (1/(1-tn_b)) * (sum_p w[b,p] g_p - x_b)

Device strategy (SPMD over 8 NeuronCores, P sharded 6250/core):
    - All matmuls are fp8e4m3 DoubleRow (157 TF/s): 3D APs [128, 2, N] whose
      middle dim holds two adjacent 128-row K-subtiles.
    - Phase 1 (over p-windows of 512): z[b,p] = x''^T.T @ G'^T with
      x'' = (alpha*XSCALE)*x as the stationary operand (2^9 prescale keeps fp8
      out of the subnormal range); ScalarE exp(scale=1/XSCALE) emits E fp8 and
      accum_out accumulates the softmax denominator s per batch row (the final
      partial window is split so pad columns never pollute s); PE transposes
      E 128x128 tiles into the resident E^T [p, b] buffer.
    - Phase 2 (mm2) runs in two d-column halves x two p-pair sets:
      W[b,d] += E^T-pair.T @ G_nat-pair accumulates a full pair-set per PSUM
      bank; the first d-half's ReduceScatter is issued while the second half
      computes, so only the second RS is exposed as tail.
    - Each core epilogues its own 32 batch rows in f32 (out ~= -x/(1-tn), so
      f32 epilogue keeps overall rel-err ~2e-4 despite fp8 matmuls).
"""

import math

import numpy as np
import ml_dtypes

import concourse.bass as bass
import concourse.bacc as bacc
import concourse.tile as tile
import concourse.mybir as mybir
from concourse import bass_utils

FP8 = mybir.dt.float8e4
F32 = mybir.dt.float32
NP_FP8 = mybir.dt.np(FP8)

T_SCHEDULE = 999.0
N_CORES = 8
XSCALE = 512.0  # power-of-2 prescale on x'' so fp8 values stay normal-range
PW = 512        # p-window width for mm1


class Cfg:
    def __init__(self, B=256, D=3072, P=50000):
        assert B % 128 == 0 and B % N_CORES == 0
        assert D % 512 == 0 and D % 256 == 0
        assert P % N_CORES == 0
        self.B = B
        self.D = D
        self.P = P
        self.PSH = P // N_CORES                       # db rows per core
        self.NW = math.ceil(self.PSH / PW)            # mm1 p-windows
        self.PCH = math.ceil(self.PSH / 256)          # p-chunk PAIRS for mm2
        self.KCH = D // 256                           # K-pairs (d-chunks / 2)
        self.NDG = D // 512                           # mm2 512-wide d-groups
        self.BC = B // 128                            # b-chunks
        self.BR = B // N_CORES                        # output rows per core
        self.NH = 2 if self.NDG % 2 == 0 else 1       # d-halves (split RS)


def build_nc(cfg: Cfg):
    nc = bacc.Bacc(
        "TRN2", target_bir_lowering=False, debug=False, num_devices=N_CORES
    )
    gT = nc.declare_dram_parameter(
        "gT", [cfg.NW, 128, cfg.KCH, 2, PW], FP8, isOutput=False
    )
    gN = nc.declare_dram_parameter(
        "gN", [cfg.PCH, 128, 2, cfg.D], FP8, isOutput=False
    )
    # mm1 stationary operand, SW-interleaved for DoubleRowSwInterleave:
    # [i, k, m, j] = x''T[(2k+j)*128+i, bchunk*128 + (127-m)]
    xT = nc.declare_dram_parameter(
        "xT", [128, cfg.KCH, cfg.B, 2], FP8, isOutput=False
    )
    xi = nc.declare_dram_parameter("xi", [cfg.BR, cfg.D], F32, isOutput=False)
    sc = nc.declare_dram_parameter("sc", [cfg.BR, 1], F32, isOutput=False)
    out = nc.declare_dram_parameter("out", [cfg.BR, cfg.D], F32, isOutput=True)

    DR = mybir.MatmulPerfMode.DoubleRow
    DRSW = mybir.MatmulPerfMode.DoubleRowSwInterleave
    EXP = mybir.ActivationFunctionType.Exp

    HC = cfg.D // cfg.NH          # columns per d-half
    HG = cfg.NDG // cfg.NH        # 512-groups per d-half
    # p-pair sets (second-level K split of mm2 so PSUM holds a full set)
    half_pairs = math.ceil(cfg.PCH / 2)
    psets = [list(range(half_pairs)), list(range(half_pairs, cfg.PCH))]
    psets = [ps for ps in psets if ps]

    ident_np = np.eye(128, dtype=NP_FP8)
    ident_dram = nc.inline_tensor(ident_np, name="ident")

    with tile.TileContext(nc) as tc:
        with (
            tc.tile_pool(name="persist", bufs=1) as pp,
            tc.tile_pool(name="gtp", bufs=3) as gtp,
            tc.tile_pool(name="gnp", bufs=2 * half_pairs + 2) as gnp,
            tc.tile_pool(name="ep", bufs=4) as epool,
            tc.tile_pool(name="stg", bufs=1) as stg,
            tc.tile_pool(name="dram", bufs=1, space="DRAM") as dram,
            tc.tile_pool(name="zps", bufs=3, space="PSUM") as zpool,
            tc.tile_pool(name="wps", bufs=2, space="PSUM") as wpool,
            tc.tile_pool(name="tps", bufs=3, space="PSUM") as tpool,
        ):
            xT_sb = pp.tile([128, cfg.KCH, cfg.B, 2], FP8)
            nc.sync.dma_start(xT_sb[:], xT[:])
            ident_sb = pp.tile([128, 128], FP8)
            nc.sync.dma_start(ident_sb[:], ident_dram[:])
            xi_sb = pp.tile([cfg.BR, cfg.D], F32)
            nc.sync.dma_start(xi_sb[:], xi[:])
            sc_sb = pp.tile([cfg.BR, 1], F32)
            nc.sync.dma_start(sc_sb[:], sc[:])

            ET = pp.tile([128, cfg.PCH, 2, cfg.B], FP8)
            s_acc = [
                pp.tile([128, 1], F32, name=f"sacc{b}", tag=f"sacc{b}")
                for b in range(cfg.BC)
            ]
            Wah = [
                pp.tile([128, HC], F32, name=f"wah{b}", tag=f"wah{b}")
                for b in range(cfg.BC)
            ]
            for b in range(cfg.BC):
                nc.vector.memset(s_acc[b][:], 0.0)

            rs_in = [
                dram.tile(
                    [cfg.B, HC + (1 if h == cfg.NH - 1 else 0)], F32,
                    name=f"rsin{h}",
                )
                for h in range(cfg.NH)
            ]
            rs_out = [
                dram.tile(
                    [cfg.BR, HC + (1 if h == cfg.NH - 1 else 0)], F32,
                    name=f"rsout{h}",
                )
                for h in range(cfg.NH)
            ]

            # ---------------- Phase 1: mm1 + exp + transposes ----------------
            for w in range(cfg.NW):
                gt_t = gtp.tile([128, cfg.KCH, 2, PW], FP8, tag="gt")
                nc.sync.dma_start(gt_t[:], gT[w])
                vc = max(0, min(cfg.PSH - w * PW, PW))
                for b in range(cfg.BC):
                    z = zpool.tile([128, PW], F32, tag="z")
                    for k in range(cfg.KCH):
                        nc.tensor.matmul(
                            z[:],
                            xT_sb[:, k, b * 128 : (b + 1) * 128, :],
                            gt_t[:, k, :, :],
                            start=(k == 0),
                            stop=(k == cfg.KCH - 1),
                            perf_mode=DRSW,
                        )
                    e_t = epool.tile([128, PW], FP8, tag="e")
                    s_part = pp.tile(
                        [128, 1], F32, name=f"sp{w}_{b}", tag="spart", bufs=4
                    )
                    if vc == PW:
                        nc.scalar.activation(
                            e_t[:], z[:], EXP, scale=1.0 / XSCALE,
                            accum_out=s_part[:],
                        )
                    else:
                        nc.scalar.activation(
                            e_t[:, :vc], z[:, :vc], EXP, scale=1.0 / XSCALE,
                            accum_out=s_part[:],
                        )
                        nc.scalar.activation(
                            e_t[:, vc:], z[:, vc:], EXP, scale=1.0 / XSCALE,
                        )
                    nc.vector.tensor_add(s_acc[b][:], s_acc[b][:], s_part[:])
                    # transpose the window's 128x128 E blocks into one PSUM
                    # tile (fp8 transpose writes with element step 2), then
                    # one batched ScalarE copy into ET
                    nv = min(PW // 128, 2 * cfg.PCH - w * (PW // 128))
                    if nv <= 0:
                        continue
                    t_ps = tpool.tile([128, PW, 2], FP8, tag="t")
                    for c in range(nv):
                        nc.tensor.transpose(
                            t_ps[:, c * 128 : (c + 1) * 128, 0],
                            e_t[:, c * 128 : (c + 1) * 128],
                            ident_sb[:],
                        )
                    src = t_ps[:, : nv * 128, 0].rearrange(
                        "p (a c) -> p a c", c=128
                    )
                    npair = (nv + 1) // 2
                    dst = ET[
                        :, 2 * w : 2 * w + npair, :, b * 128 : (b + 1) * 128
                    ].rearrange("p a b c -> p (a b) c")[:, :nv, :]
                    nc.scalar.copy(dst, src)

            # ------------- Phase 2: mm2 per d-half, split RS -------------
            for h in range(cfg.NH):
                c0 = h * HC
                for si, pset in enumerate(psets):
                    gn_tiles = {}
                    for m in pset:
                        gn_t = gnp.tile([128, 2, HC], FP8, tag="gn")
                        nc.sync.dma_start(gn_t[:], gN[m, :, :, c0 : c0 + HC])
                        gn_tiles[m] = gn_t
                    for b in range(cfg.BC):
                        for dg in range(HG):
                            wp = wpool.tile([128, 512], F32, tag="w")
                            for j, m in enumerate(pset):
                                nc.tensor.matmul(
                                    wp[:],
                                    ET[:, m, :, b * 128 : (b + 1) * 128],
                                    gn_tiles[m][
                                        :, :, dg * 512 : (dg + 1) * 512
                                    ],
                                    start=(j == 0),
                                    stop=(j == len(pset) - 1),
                                    perf_mode=DR,
                                )
                            dst = Wah[b][:, dg * 512 : (dg + 1) * 512]
                            if si == 0:
                                nc.scalar.copy(dst, wp[:])
                            else:
                                nc.vector.tensor_add(dst, dst, wp[:])
                            if si == len(psets) - 1:
                                nc.sync.dma_start(
                                    rs_in[h][
                                        b * 128 : (b + 1) * 128,
                                        dg * 512 : (dg + 1) * 512,
                                    ],
                                    dst,
                                )
                if h == cfg.NH - 1:
                    for b in range(cfg.BC):
                        nc.sync.dma_start(
                            rs_in[h][b * 128 : (b + 1) * 128, HC : HC + 1],
                            s_acc[b][:],
                        )
                nc.gpsimd.collective_compute(
                    "ReduceScatter",
                    mybir.AluOpType.add,
                    replica_groups=[list(range(N_CORES))],
                    ins=[rs_in[h].opt()],
                    outs=[rs_out[h].opt()],
                )

            # ---------------------- epilogue ----------------------
            eps = []
            for h in range(cfg.NH):
                w_ = HC + (1 if h == cfg.NH - 1 else 0)
                e = stg.tile([cfg.BR, w_], F32, name=f"eph{h}", tag=f"eph{h}")
                nc.sync.dma_start(e[:], rs_out[h][:])
                eps.append(e)
            rec = pp.tile([cfg.BR, 1], F32)
            nc.vector.reciprocal(rec[:], eps[-1][:, HC : HC + 1])
            nc.vector.tensor_mul(rec[:], rec[:], sc_sb[:])
            out_sb = pp.tile([cfg.BR, cfg.D], F32)
            for h in range(cfg.NH):
                nc.vector.scalar_tensor_tensor(
                    out_sb[:, h * HC : h * HC + HC],
                    eps[h][:, :HC],
                    rec[:],
                    xi_sb[:, h * HC : h * HC + HC],
                    op0=mybir.AluOpType.mult,
                    op1=mybir.AluOpType.subtract,
                )
            nc.sync.dma_start(out[:], out_sb[:])

    nc.compile()
    return nc


def prep_in_maps(cfg: Cfg, xt, t, gt_images):
    B, D, P = cfg.B, cfg.D, cfg.P
    x = np.asarray(xt, dtype=np.float32).reshape(B, -1)
    g = np.asarray(gt_images, dtype=np.float32).reshape(P, -1)
    t = np.asarray(t, dtype=np.float32).reshape(B)
    assert x.shape[1] == D

    tn = t / T_SCHEDULE
    inv_var = 1.0 / (2.0 * (1.0 - tn) ** 2)
    alpha = 2.0 * inv_var * tn
    inv1mtn = 1.0 / (1.0 - tn)

    # x''^T pretiled + SW-interleaved for DoubleRowSwInterleave:
    # [128, KCH, B, 2]: [i, k, bc*128+m, j] = x''T[(2k+j)*128+i, bc*128+127-m]
    xp = (x * (alpha * XSCALE)[:, None]).T  # [D, B]
    x4 = xp.reshape(cfg.KCH, 2, 128, B // 128, 128)   # [k, j, i, bc, n]
    x4 = x4[:, :, :, :, ::-1]                          # reverse cols in chunk
    xT_tiled = np.ascontiguousarray(
        x4.transpose(2, 0, 3, 4, 1).reshape(128, cfg.KCH, B, 2)
    ).astype(NP_FP8)

    PWTOT = cfg.NW * PW
    in_maps = []
    for c in range(N_CORES):
        gs = g[c * cfg.PSH : (c + 1) * cfg.PSH]
        # G'^T padded [D, PWTOT] -> [NW, 128, KCH, 2, PW]:
        # [w, i, k, j, p'] = G'T[(2k+j)*128+i, w*PW+p']
        gtp_ = np.zeros((D, PWTOT), np.float32)
        gtp_[:, : cfg.PSH] = gs.T
        gTb = np.ascontiguousarray(
            gtp_.reshape(cfg.KCH, 2, 128, cfg.NW, PW).transpose(3, 2, 0, 1, 4)
        ).astype(NP_FP8)
        # G_nat pair tiles [PCH, 128, 2, D]: [m, i, j, d] = G[(2m+j)*128+i, d]
        gn_ = np.zeros((cfg.PCH * 256, cfg.D), np.float32)
        gn_[: cfg.PSH] = gs
        gNb = np.ascontiguousarray(
            gn_.reshape(cfg.PCH, 2, 128, cfg.D).transpose(0, 2, 1, 3)
        ).astype(NP_FP8)
        rows = slice(c * cfg.BR, (c + 1) * cfg.BR)
        xi = np.ascontiguousarray(x[rows] * inv1mtn[rows, None]).astype(np.float32)
        sc = np.ascontiguousarray(inv1mtn[rows, None]).astype(np.float32)
        in_maps.append({"gT": gTb, "gN": gNb, "xT": xT_tiled, "xi": xi, "sc": sc})
    return in_maps


_NC_CACHE = {}


def _get_nc(cfg: Cfg):
    key = (cfg.B, cfg.D, cfg.P)
    if key not in _NC_CACHE:
        _NC_CACHE[key] = build_nc(cfg)
    return _NC_CACHE[key]


def kernel(xt, t, gt_images, _trace=False):
    xt = np.asarray(xt)
    cfg = Cfg(B=xt.shape[0], D=int(np.prod(xt.shape[1:])),
              P=np.asarray(gt_images).shape[0])
    nc = _get_nc(cfg)
    in_maps = prep_in_maps(cfg, xt, t, gt_images)
    res = bass_utils.run_bass_kernel_spmd(
        nc, in_maps, core_ids=list(range(N_CORES)), trace=_trace
    )
    out = np.concatenate(
        [res.results[c]["out"] for c in range(N_CORES)], axis=0
    ).astype(np.float32)
    if _trace:
        kernel.last_exec_time_ns = res.exec_time_ns
    return out.reshape(xt.shape)
